# revision 30
# baseline (speedup 1.0000x reference)
"""Causal attention block (B=2, S=2048, H=1024, 16 heads) on 8 NeuronCores.

Sharding: core c handles batch b = c // 4 and head-group g = c % 4
(4 heads = 256 qkv columns / w_out rows per core). Each core computes a
partial output y_partial = softmax(QK^T/sqrt(d)) V @ Wout_slice for its
heads; the host sums the 4 head-group partials per batch.

On-chip layout (per core):
  x^T   [H=1024, S=2048]  (host-transposed)   - h on partitions
  Q^T,K^T as two head-PAIR tiles [128, 2048]: partitions 0-63 head 2p,
        64-127 head 2p+1 (d on partitions)    - from matmul(W, x^T)
  S^T = K^T.T @ Q^T per (t-chunk 128, s-chunk 512), row-tiled 2 heads
        concurrently on the PE (K=64 each)
  softmax without max-subtraction (scores are O(10), exp is safe in f32);
        causal masking via additive -1e38 mask (built on-chip with
        affine_select) added into the PSUM triangular band before a single
        exp per block; fully-masked columns are never computed (narrower
        matmuls / exps; PSUM accumulation is per-element so this is exact)
  PV: out^T accumulation with V augmented by a ones column, which makes
        the denominator Z land in an extra PSUM row for free
  normalize: DVE reciprocal -> PE K=1 outer-product broadcast -> DVE mul
        (gpsimd partition_broadcast and 1-partition custom-DVE ops are
        broken on this hardware; DMA rejects partition-step-0 APs)
  out-proj: y = V~^T.T @ Wout per s-tile, accumulated over 2 pairs;
        j=3 borrows the idle qkv PSUM banks for deeper S^T/out-proj
        pipelining in the ACT-bound causal tail

All matmuls use fp32r (4-byte, ~tf32 precision, 1 cycle/row at N>=256).
"""

import numpy as np
from contextlib import ExitStack

import concourse.bass as bass
import concourse.tile as tile
import concourse.mybir as mybir
from concourse import bacc
from concourse import bass_utils

F32 = mybir.dt.float32
F32R = mybir.dt.float32r
AF = mybir.ActivationFunctionType

B, S, H = 2, 2048, 1024
NH, DH = 16, 64
NCORES = 8
SC = 512            # s-chunk width
NSC = S // SC       # 4
NTC = S // 128      # 16 t-chunks
NHC = H // 128      # 8 h contraction chunks

_CACHE = {}


def _build():
    nc = bacc.Bacc("TRN2", target_bir_lowering=False, debug=False,
                   enable_asserts=False, num_devices=NCORES)
    xT = nc.dram_tensor("xT", [H, S], F32, kind="ExternalInput").ap()
    wq = nc.dram_tensor("wq", [H, 256], F32, kind="ExternalInput").ap()
    wk = nc.dram_tensor("wk", [H, 256], F32, kind="ExternalInput").ap()
    wv = nc.dram_tensor("wv", [H, 256], F32, kind="ExternalInput").ap()
    wo = nc.dram_tensor("wo", [256, H], F32, kind="ExternalInput").ap()
    mask = nc.dram_tensor("mask", [4, 128, SC], F32, kind="ExternalInput").ap()
    vaug = nc.dram_tensor("vaug", [128, 130], F32, kind="ExternalInput").ap()
    ones = nc.dram_tensor("ones", [128, 128], F32, kind="ExternalInput").ap()
    y = nc.dram_tensor("y", [S, H], F32, kind="ExternalOutput").ap()

    with tile.TileContext(nc) as tc:
        with ExitStack() as ctx:
            pw = ctx.enter_context(tc.tile_pool(name="w", bufs=1))
            pxt = ctx.enter_context(tc.tile_pool(name="xt", bufs=2))
            pbig = ctx.enter_context(tc.tile_pool(name="big", bufs=1))
            ppt = ctx.enter_context(tc.tile_pool(name="pt", bufs=6))
            pzz = ctx.enter_context(tc.tile_pool(name="zz", bufs=3))
            pyo = ctx.enter_context(tc.tile_pool(name="yo", bufs=4))
            import os
            _b = os.environ.get("KBUFS", "2,3,2,1").split(",")
            bq, bs, bp, by = (int(v) for v in _b)  # PSUM banks: qkv/s/pv/y
            ps_qkv = ctx.enter_context(tc.tile_pool(name="psqkv", bufs=bq, space="PSUM"))
            ps_s = ctx.enter_context(tc.tile_pool(name="pss", bufs=bs, space="PSUM"))
            ps_pv = ctx.enter_context(tc.tile_pool(name="pspv", bufs=bp, space="PSUM"))
            ps_y = ctx.enter_context(tc.tile_pool(name="psy", bufs=by, space="PSUM"))

            # ---- weights & masks (scalar-engine DGE queue; sync queue
            #      carries the x^T / y traffic) ----
            wq_t, wk_t, wv_t = [], [], []
            for hc in range(NHC):
                t = pw.tile([128, 256], F32R, tag=f"wq{hc}", name=f"wq{hc}")
                nc.sync.dma_start(
                    t[:], wq[hc * 128:(hc + 1) * 128, :].bitcast(F32R))
                wq_t.append(t)
            mask_all0 = pw.tile([128, 4 * SC], F32, tag="mask", name="mask_all")
            nc.sync.dma_start(
                mask_all0[:].rearrange("p (k s) -> p k s", k=4),
                mask.rearrange("k p s -> p k s"))
            for hc in range(NHC):
                t = pw.tile([128, 256], F32R, tag=f"wv{hc}", name=f"wv{hc}")
                nc.sync.dma_start(
                    t[:], wv[hc * 128:(hc + 1) * 128, :].bitcast(F32R))
                wv_t.append(t)
            # wk / masks / vaug / wo are loaded later (inside the j-loop)
            # so the x^T chunk transfers win shared HBM bandwidth first.
            wo_t, mask_t = [], []

            # ---- persistent activations ----
            QT = [pbig.tile([128, S], F32R, tag=f"qt{p}", name=f"qt{p}") for p in range(2)]
            KT = [pbig.tile([128, S], F32R, tag=f"kt{p}", name=f"kt{p}") for p in range(2)]
            VT = [pbig.tile([128, S], F32R, tag=f"vt{p}", name=f"vt{p}") for p in range(2)]
            # V_aug per t-chunk, grouped per head pair (193 cols each):
            # even head-local: [V(64) | 1]         -> out rows 0..64, Z row 64
            # odd  head-local: [zeros(32) | 1 | zeros(31) | V] -> out rows 0..127
            #                  (base 0), Z row 32, V~ rows 64..127
            VA = [pbig.tile([128, 386], F32R, tag=f"va{t_}", name=f"va{t_}") for t_ in range(NTC)]

            for j in range(NSC):
                sj = slice(j * SC, (j + 1) * SC)
                # ---- load x^T column-block j (two 1 MB halves so the
                #      hc=0..3 accumulation can start while 4..7 streams) ----
                xt_all = pxt.tile([128, NHC * SC], F32R, tag="xt",
                                  name=f"xt{j}")
                xt_src = xT.rearrange("(c p) s -> p c s", p=128)[:, :, sj]
                xt_dst = xt_all[:].rearrange("p (c s) -> p c s", c=NHC)
                nsplit = 4 if j == 0 else 2
                step = NHC // nsplit
                for si in range(nsplit):
                    nc.sync.dma_start(
                        xt_dst[:, si * step:(si + 1) * step, :],
                        xt_src[:, si * step:(si + 1) * step, :].bitcast(F32R))
                xt_j = [xt_all[:, hc * SC:(hc + 1) * SC] for hc in range(NHC)]

                if j == 0:
                    for hc in range(NHC):
                        t = pw.tile([128, 256], F32R, tag=f"wk{hc}",
                                    name=f"wk{hc}")
                        nc.sync.dma_start(
                            t[:], wk[hc * 128:(hc + 1) * 128, :].bitcast(F32R))
                        wk_t.append(t)
                    mask_t = [mask_all0[:, k * SC:(k + 1) * SC]
                              for k in range(4)]
                    ones_t = pw.tile([128, 128], F32R, tag="ones")
                    nc.sync.dma_start(ones_t[:], ones[:].bitcast(F32R))
                    vaug_sb = pw.tile([128, 130], F32R, tag="vaug")
                    nc.sync.dma_start(vaug_sb[:], vaug[:].bitcast(F32R))
                # ---- Q^T / K^T for s-chunk j ----
                for p in range(2):
                    for W, OUT in ((wq_t, QT), (wk_t, KT)):
                        ps = ps_qkv.tile([128, SC], F32, tag="qkv")
                        for hc in range(NHC):
                            nc.tensor.matmul(
                                ps[:], W[hc][:, p * 128:(p + 1) * 128],
                                xt_j[hc],
                                start=(hc == 0), stop=(hc == NHC - 1))
                        nc.vector.tensor_copy(OUT[p][:, sj], ps[:])

                # ---- V for t-chunks 4j..4j+3 ----
                for tci in range(4):
                    t_ = 4 * j + tci
                    ps = ps_qkv.tile([128, 256], F32, tag="qkv")
                    for hc in range(NHC):
                        nc.tensor.matmul(
                            ps[:],
                            xt_all[:, hc * SC + tci * 128:
                                   hc * SC + (tci + 1) * 128],
                            wv_t[hc][:], start=(hc == 0), stop=(hc == NHC - 1))
                    va3 = VA[t_][:].rearrange("p (g c) -> p g c", c=193)
                    psv3 = ps[:].rearrange("p (g c) -> p g c", c=128)
                    nc.vector.tensor_copy(va3[:, :, 0:64], psv3[:, :, 0:64])
                    nc.vector.tensor_copy(va3[:, :, 129:193], psv3[:, :, 64:128])
                    nc.vector.tensor_copy(
                        va3[:, :, 64:129],
                        vaug_sb[:].rearrange("p (g c) -> p g c", c=65))

                # ---- attention for s-chunk j ----
                ntc = 4 * j + 4
                for p in range(2):
                    pp = {}
                    for r in range(2):
                        pp[r] = ps_pv.tile([128, SC], F32, tag="pv", name=f"pv{p}_{r}")
                    for tcc in range(ntc):
                        # diagonal blocks only touch s-columns >= 128k
                        # (k = position within the diagonal 512x512 square);
                        # cols < 128k are fully masked and never computed.
                        if tcc >= 4 * j:
                            k = tcc - 4 * j
                            c0 = 128 * k          # valid col start
                            c1 = 128 * (k + 1)    # end of triangular band
                        else:
                            k, c0, c1 = None, 0, 0
                        sjv = slice(j * SC + c0, (j + 1) * SC)
                        pts = {}
                        for r in range(2):
                            pool_s = (ps_qkv if (j == 3 and (tcc + r) % 2 == 0)
                                      else ps_s)
                            ss = pool_s.tile([128, SC], F32,
                                             tag="qkv" if pool_s is ps_qkv
                                             else "s", name=f"ss{r}")
                            nc.tensor.matmul(
                                ss[:, c0:SC],
                                KT[p][64 * r:64 * (r + 1),
                                      tcc * 128:(tcc + 1) * 128],
                                QT[p][64 * r:64 * (r + 1), sjv],
                                start=True, stop=True)
                            pt = ppt.tile([128, SC], F32R, tag="pt")
                            if k is not None:
                                # triangular 128-col band: exp in psum, mask
                                nc.scalar.activation(ss[:, c0:c1],
                                                     ss[:, c0:c1], AF.Exp)
                                nc.vector.tensor_mul(pt[:, c0:c1],
                                                     ss[:, c0:c1],
                                                     mask_t[k][:, c0:c1])
                                if c1 < SC:  # fully-visible remainder
                                    nc.scalar.activation(pt[:, c1:SC],
                                                         ss[:, c1:SC], AF.Exp)
                            else:
                                nc.scalar.activation(pt[:], ss[:], AF.Exp)
                            pts[r] = pt
                        for r in range(2):
                            if r == 0:
                                out_sl = pp[r][0:65, c0:SC]
                                lhs_sl = VA[tcc][:, 193 * p:193 * p + 65]
                            else:
                                out_sl = pp[r][0:128, c0:SC]
                                lhs_sl = VA[tcc][:, 193 * p + 65:193 * p + 193]
                            nc.tensor.matmul(
                                out_sl, lhs_sl, pts[r][:, c0:SC],
                                start=(tcc == 0), stop=(tcc == ntc - 1))
                    # normalize: V~^T = PV / Z
                    # recip (DVE) -> PE outer-product broadcast -> copy -> mul
                    for r in range(2):
                        z_row = 64 if r == 0 else 32
                        zr = pzz.tile([65, SC], F32R, tag="zr")
                        with nc.allow_low_precision(reason="f32r recip feeds bcast matmul"):
                            nc.vector.reciprocal(
                                zr[z_row:z_row + 1, :], pp[r][z_row:z_row + 1, :])
                        rbp = ps_y.tile([128, SC], F32, tag="y",
                                        name=f"rbp{p}_{r}")
                        nc.tensor.matmul(rbp[:], ones_t[z_row:z_row + 1, :],
                                         zr[z_row:z_row + 1, :],
                                         start=True, stop=True)
                        rb = pzz.tile([128, SC], F32, tag="rb")
                        if r == 0:
                            rb_sl, v_sl = rb[0:64, :], pp[r][0:64, :]
                        else:
                            rb_sl, v_sl = rb[64:128, :], pp[r][64:128, :]
                        nc.vector.tensor_copy(rb_sl, rbp[0:64, :] if r == 0
                                              else rbp[64:128, :])
                        if j == 3:
                            # 128-col slices so the tail out-proj can start
                            # on the first s-tile before the rest normalize
                            for q4 in range(4):
                                qs = slice(q4 * 128, (q4 + 1) * 128)
                                nc.vector.tensor_mul(
                                    VT[p][64 * r:64 * (r + 1),
                                          j * SC + q4 * 128:
                                          j * SC + (q4 + 1) * 128],
                                    v_sl[:, qs], rb_sl[:, qs])
                        else:
                            nc.vector.tensor_mul(
                                VT[p][64 * r:64 * (r + 1), sj], v_sl, rb_sl)

                # ---- out-projection for s-tiles in chunk j ----
                if j == 0:
                    for p in range(2):
                        t = pw.tile([128, H], F32R, tag=f"wo{p}",
                                    name=f"wo{p}")
                        nc.sync.dma_start(
                            t[:], wo[p * 128:(p + 1) * 128, :].bitcast(F32R))
                        wo_t.append(t)
                for sti in range(4):
                    st = 4 * j + sti
                    ysb = pyo.tile([128, H], F32, tag="y", name=f"ysb{st}")
                    for n2 in range(2):
                        pool_y = ps_qkv if (j == 3 and n2 == 1) else ps_y
                        py_ = pool_y.tile([128, 512], F32,
                                          tag="qkv" if pool_y is ps_qkv
                                          else "y", name=f"py{sti}_{n2}")
                        for p in range(2):
                            nc.tensor.matmul(
                                py_[:], VT[p][:, st * 128:(st + 1) * 128],
                                wo_t[p][:, n2 * 512:(n2 + 1) * 512],
                                start=(p == 0), stop=(p == 1))
                        nc.any.tensor_copy(
                            ysb[:, n2 * 512:(n2 + 1) * 512], py_[:])
                    if j == 3:
                        nc.sync.dma_start(
                            y[st * 128:(st + 1) * 128, 0:512], ysb[:, 0:512])
                        nc.sync.dma_start(
                            y[st * 128:(st + 1) * 128, 512:H], ysb[:, 512:H])
                    else:
                        nc.sync.dma_start(y[st * 128:(st + 1) * 128, :],
                                          ysb[:])
    nc.compile()
    return nc


def _masks():
    k = np.arange(4)[:, None, None]
    p = np.arange(128)[None, :, None]
    f = np.arange(SC)[None, None, :]
    return (f >= 128 * k + p).astype(np.float32)


def _in_maps(x, w_qkv, w_out):
    x = np.asarray(x, dtype=np.float32)
    w_qkv = np.asarray(w_qkv, dtype=np.float32)
    w_out = np.asarray(w_out, dtype=np.float32)
    mask = _masks()
    vaug_const = np.zeros((128, 130), dtype=np.float32)
    vaug_const[:, 0] = 1.0      # even-head ones col (group col 64)
    vaug_const[:, 33] = 1.0     # odd-head ones col (group col 97)
    vaug_const[:, 65] = 1.0
    vaug_const[:, 98] = 1.0
    ones_const = np.ones((128, 128), dtype=np.float32)
    scale = np.float32(1.0 / np.sqrt(DH))
    in_maps = []
    for c in range(NCORES):
        b, g = divmod(c, 4)
        cols = slice(256 * g, 256 * (g + 1))
        in_maps.append({
            "xT": np.ascontiguousarray(x[b].T),
            "wq": np.ascontiguousarray(w_qkv[:, 0 * H:1 * H][:, cols]) * scale,
            "wk": np.ascontiguousarray(w_qkv[:, 1 * H:2 * H][:, cols]),
            "wv": np.ascontiguousarray(w_qkv[:, 2 * H:3 * H][:, cols]),
            "wo": np.ascontiguousarray(w_out[cols, :]),
            "mask": mask,
            "vaug": vaug_const,
            "ones": ones_const,
        })
    return in_maps


TRACE = False
LAST_RESULTS = None


def kernel(x, w_qkv, w_out):
    global LAST_RESULTS
    if "nc" not in _CACHE:
        _CACHE["nc"] = _build()
    nc = _CACHE["nc"]
    in_maps = _in_maps(x, w_qkv, w_out)
    res = bass_utils.run_bass_kernel_spmd(
        nc, in_maps, core_ids=list(range(NCORES)), trace=TRACE)
    LAST_RESULTS = res
    y = np.zeros((B, S, H), dtype=np.float32)
    for c in range(NCORES):
        y[c // 4] += res.results[c]["y"]
    return y
